# revision 24
# baseline (speedup 1.0000x reference)
"""Distributed GQA attention block (RMSNorm-QK + RoPE + causal attention + Wo)
for one TRN2 chip (8 NeuronCores).

Sharding: tensor-parallel over heads. Core i computes q-heads {2i, 2i+1} and
kv-head i//2. Everything on-device is computed transposed ([dim, seq]) so the
hidden/contraction axis lands on SBUF partitions with zero on-device
transposes of X. An AllToAll redistributes the attention output from
head-sharded to sequence-sharded; each core then runs the output projection
for its own 256 tokens. The final RMSNorm is a pure per-token scale, so it is
applied host-side (exactly), together with last_norm_scale.

v8 structure (from v3 via trace-driven iteration; 303us -> ~217us):
- X^T streamed per 512-column chunk through 2 rotating SBUF buffers; Wo is
  resident in its own SBUF space and loaded early on the sync queue, so no
  WAR alias with X and no HBM spike colliding with the AllToAlls.
- Startup split: wq + rope tables pull on the scalar ring while x chunks pull
  on the sync ring; first projection matmul starts ~10us in (8us preamble).
- RMS stats use Ln->Exp (ri = exp(-0.5*ln(m))) on the scalar engine, plus an
  act-table patch steering both onto the one table set that holds ln AND exp
  (natural_log_exp_and_others): activation-table reloads drop 25 -> 2.
- Per-token 1/sqrt broadcasts ([1,CW] -> [128,CW]) ride PE outer products in
  bf16 (1 cycle/row); ri is produced in bf16 directly by the Exp activation.
- Softmax denominator accumulated in bf16 (2x DVE rate); denominator errors
  are per-token scales that the final (host-side) RMSNorm cancels exactly.
- V tiles transposed on the PE (identity matmul) instead of DMA-transpose.
- A tiny warmup AllToAll at kernel start absorbs the first-collective ncfw
  bootstrap (~50us "Invalid" CC op + ~11us trigger->start delay).
- Head-1 attention chunks are interleaved right after their head-0 siblings
  (both ready at the same time), except (1,3): the head-0 AllToAll fires
  ~12us before attention ends and overlaps attn(1,3).
- a2a_out readbacks ride the scalar queue AFTER all exps (structurally
  impossible for phase-D matmuls to be queued ahead of attention tail work),
  per 64KB g-block.
- Phase D uses all 8 PSUM banks: even heads (post a2a#0) accumulate while
  a2a#1 flies; odd-head accumulation continues in the same banks, split by
  token-half so the first half's stores overlap the second half's matmuls.
- fp8 was tried and rejected: with random-sign values the Wo contraction and
  the p-weighted V average inherit elementwise quantization error ~1:1, so
  e4m3's ~4-5% blows the 2e-2 gate (measured 6.5e-2 proj / 3.8e-2 AV).

Numerics: bf16 matmuls with f32 PSUM accumulation; softmax without
max-subtraction (logits are O(1)); causal mask applied multiplicatively
after exp; K normalization folds the 1/sqrt(D) logit scale.
"""

import sys

sys.path.insert(0, "/opt/trn_rl_repo")

import numpy as np
import ml_dtypes

BF16 = ml_dtypes.bfloat16

S = 2048  # sequence length
H = 2048  # hidden
D = 128  # head dim
NH = 16  # query heads
NKV = 4  # kv heads
NC = 8  # cores
HL = NH // NC  # q heads per core = 2
SC = S // NC  # seq per core (output shard) = 256
CH = 4  # seq chunks
CW = 512  # chunk width
KT = H // 128  # contraction tiles = 16
EPS = 1e-6

_cache: dict = {}


def _patch_act_tables():
    """Steer Ln/Exp activations onto the one table set that holds BOTH
    (natural_log_exp_and_others), so the scalar engine never reloads its
    activation table mid-kernel. Set positions (= act_func_set ids) are
    unchanged; only which sets advertise Ln/Exp to the selection pass."""
    from concourse import hw_specs, bacc as _bacc, bass_interp as _bi

    if getattr(hw_specs, "_ant_lnexp_patch", False):
        return
    orig = hw_specs.get_activation_tables

    def patched(arch):
        tabs = orig(arch)
        both = None
        for name, fns in tabs.items():
            names = {f.name for f in fns}
            if "Exp" in names and "Ln" in names:
                both = name
                break
        if both is not None:
            for name in list(tabs):
                if name != both:
                    tabs[name] = {
                        f for f in tabs[name] if f.name not in ("Exp", "Ln")
                    }
        return tabs

    hw_specs.get_activation_tables = patched
    hw_specs._ant_lnexp_patch = True
    for mod in (_bacc, _bi):
        if getattr(mod, "get_activation_tables", None) is not None:
            mod.get_activation_tables = patched


def _build_nc(reps: int = 1):
    import concourse.bass as bass
    import concourse.tile as tile
    from concourse import bacc, mybir

    _patch_act_tables()

    f32 = mybir.dt.float32
    f32r = mybir.dt.float32r
    bf16 = mybir.dt.bfloat16
    AF = mybir.ActivationFunctionType

    nc = bacc.Bacc("TRN2", target_bir_lowering=False, debug=False, num_devices=NC)

    # ---- kernel I/O (per-core shards; replicated where noted) ----
    xt_d = nc.dram_tensor("xt", [128, KT, S], bf16, kind="ExternalInput").ap()
    wq_d = nc.dram_tensor("wq", [128, KT, HL * D], bf16, kind="ExternalInput").ap()
    wk_d = nc.dram_tensor("wk", [128, KT, D], bf16, kind="ExternalInput").ap()
    wv_d = nc.dram_tensor("wv", [128, KT, D], bf16, kind="ExternalInput").ap()
    wo_d = nc.dram_tensor("wo", [H, H], bf16, kind="ExternalInput").ap()
    cq_d = nc.dram_tensor("cq", [D, S], bf16, kind="ExternalInput").ap()
    sq_d = nc.dram_tensor("sq", [D, S], bf16, kind="ExternalInput").ap()
    ck_d = nc.dram_tensor("ck", [D, S], bf16, kind="ExternalInput").ap()
    sk_d = nc.dram_tensor("sk", [D, S], bf16, kind="ExternalInput").ap()
    msk_d = nc.dram_tensor("msk", [D, 4, CW], bf16, kind="ExternalInput").ap()
    id_d = nc.dram_tensor("ident", [128, 128], bf16, kind="ExternalInput").ap()
    out_d = nc.dram_tensor("out", [SC, H], bf16, kind="ExternalOutput").ap()

    with tile.TileContext(nc) as tc:
        with (
            tc.tile_pool(name="singles", bufs=1) as singles,
            tc.tile_pool(name="xs", bufs=2) as xs,  # streamed X chunks
            tc.tile_pool(name="work", bufs=3) as work,
            tc.tile_pool(name="small", bufs=3) as small,
            tc.tile_pool(name="psum", bufs=1, space="PSUM") as pp,
            tc.tile_pool(name="psmall", bufs=1, space="PSUM") as pps,
            tc.tile_pool(name="dram", bufs=1, space="DRAM") as dram,
        ):
            # ---------- resident SBUF tensors ----------
            wq_sb = singles.tile([128, KT, HL * D], bf16)
            wk_sb = singles.tile([128, KT, D], bf16)
            wv_sb = singles.tile([128, KT, D], bf16)
            cq_sb = singles.tile([128, S], bf16)
            sq_sb = singles.tile([128, S], bf16)
            ck_sb = singles.tile([128, S], bf16)
            sk_sb = singles.tile([128, S], bf16)
            msk_sb = singles.tile([128, 4, CW], bf16)
            id_sb = singles.tile([128, 128], bf16)
            wo_sb = singles.tile([128, KT, S], bf16)

            # x chunk stream: 2 rotating buffers
            x_c = [
                xs.tile([128, KT, CW], bf16, tag="xc", name=f"xc{c}")
                for c in range(CH)
            ]
            # scalar queue: wq + rope tables (idle at start); sync queue: x
            # chunks, wk/wv, wo; both rings pull in parallel so the first
            # projection matmul can start ~4us in.
            for gs in [slice(0, 2), slice(2, 4), slice(4, 8), slice(8, 12), slice(12, 16)]:
                nc.scalar.dma_start(wq_sb[:, gs, :], wq_d[:, gs, :])
            for tsl in [slice(0, 2), slice(2, 4), slice(4, 8), slice(8, 12), slice(12, 16)]:
                nc.sync.dma_start(x_c[0][:, tsl, :], xt_d[:, tsl, 0:CW])
            nc.scalar.dma_start(cq_sb, cq_d)
            nc.scalar.dma_start(sq_sb, sq_d)
            nc.scalar.dma_start(ck_sb, ck_d)
            nc.scalar.dma_start(sk_sb, sk_d)
            for g in range(4):
                gs = slice(4 * g, 4 * g + 4)
                nc.sync.dma_start(wk_sb[:, gs, :], wk_d[:, gs, :])
                nc.sync.dma_start(wv_sb[:, gs, :], wv_d[:, gs, :])
            for tp in range(4):
                tsl = slice(4 * tp, 4 * tp + 4)
                nc.sync.dma_start(x_c[1][:, tsl, :], xt_d[:, tsl, CW : 2 * CW])
            nc.sync.dma_start(msk_sb, msk_d)
            nc.sync.dma_start(id_sb, id_d)
            for tp in range(4):
                tsl = slice(4 * tp, 4 * tp + 4)
                nc.sync.dma_start(x_c[2][:, tsl, :], xt_d[:, tsl, 2 * CW : 3 * CW])
            for tp in range(4):
                tsl = slice(4 * tp, 4 * tp + 4)
                nc.sync.dma_start(x_c[3][:, tsl, :], xt_d[:, tsl, 3 * CW : 4 * CW])
            for ht in range(KT):
                nc.sync.dma_start(
                    wo_sb[:, ht, :], wo_d[ht * 128 : (ht + 1) * 128, :]
                )

            ones_col_bf = singles.tile([128, 1], bf16)  # lhsT for partition sums
            nc.vector.memset(ones_col_bf, 1.0)
            ones_row_bf = singles.tile([1, 128], bf16)  # lhsT for row broadcasts
            nc.vector.memset(ones_row_bf, 1.0)
            eps_row = singles.tile([1, 1], f32)  # D*eps for k sumsq
            nc.vector.memset(eps_row, D * EPS)
            eps_one = singles.tile([1, 1], f32)  # eps for q mean-sumsq
            nc.vector.memset(eps_one, EPS)

            qt_sb = singles.tile([128, HL, S], bf16)  # roped Q^T per local head
            kt_sb = singles.tile([128, S], bf16)  # normalized+roped K^T
            v_sb = singles.tile([128, KT, D], bf16)  # V tiles [j-part, jt, d]
            att_sb = singles.tile([128, HL, NC, SC], bf16)  # att^T post-A2A
            a2a_in = [
                dram.tile([S // 2, SC], bf16, name=f"a2ai{j}") for j in range(HL)
            ]
            a2a_out = [
                dram.tile([S // 2, SC], bf16, name=f"a2ao{j}") for j in range(HL)
            ]
            warm_in = dram.tile([NC, 16], bf16, name="warm_in")
            warm_out = dram.tile([NC, 16], bf16, name="warm_out")
            warm2_in = dram.tile([NC * 128, 32], bf16, name="warm2_in")
            warm2_out = dram.tile([NC * 128, 32], bf16, name="warm2_out")

            assert reps == 1

            # ---------- phase A: QKV projection + RMS-stats + RoPE ----------
            def phase_a(c):
                csl = slice(c * CW, (c + 1) * CW)
                xc = x_c[c]
                q_ps = [
                    pp.tile([128, CW], f32, tag="big", name=f"qps{j}", bufs=3)
                    for j in range(HL)
                ]
                for ht in range(KT):
                    mm = dict(start=(ht == 0), stop=(ht == KT - 1))
                    for j in range(HL):
                        nc.tensor.matmul(
                            q_ps[j], wq_sb[:, ht, j * D : (j + 1) * D],
                            xc[:, ht, :], **mm
                        )
                raws, rotrs = [], []

                def raw_rot(ps, name):
                    raw = work.tile([128, CW], bf16, tag="raw", bufs=4, name=name)
                    nc.vector.tensor_copy(raw, ps)
                    rotr = work.tile([128, CW], bf16, tag="rotr", bufs=3)
                    nc.gpsimd.dma_start(rotr[0:64, :], raw[64:128, :])
                    nc.gpsimd.dma_start(rotr[64:128, :], raw[0:64, :])
                    raws.append(raw)
                    rotrs.append(rotr)

                raw_rot(q_ps[0], "rawq0")
                raw_rot(q_ps[1], "rawq1")
                k_ps = pp.tile([128, CW], f32, tag="big", bufs=3)
                v_ps = pp.tile([128, CW], f32, tag="big", bufs=3)
                for ht in range(KT):
                    mm = dict(start=(ht == 0), stop=(ht == KT - 1))
                    nc.tensor.matmul(k_ps, wk_sb[:, ht, :], xc[:, ht, :], **mm)
                    nc.tensor.matmul(v_ps, wv_sb[:, ht, :], xc[:, ht, :], **mm)
                raw_rot(k_ps, "rawk")
                # V: copy psum -> SBUF (transposed layout), then PE-transpose
                # each 128x128 tile into v_sb
                vt = work.tile([128, CW], bf16, tag="vt", bufs=2, name="vt")
                nc.vector.tensor_copy(vt, v_ps)
                for t_ in range(4):
                    jt = 4 * c + t_
                    vT_ps = pp.tile([128, 128], bf16, tag="st", bufs=2, name="vTps")
                    nc.tensor.transpose(
                        vT_ps, vt[:, t_ * 128 : (t_ + 1) * 128], id_sb
                    )
                    nc.vector.tensor_copy(v_sb[:, jt, :], vT_ps)
                # RMS stats: ri = exp(-0.5 * ln(m)); ln+exp share one ACT table
                ris = []
                for idx in range(3):
                    raw = raws[idx]
                    sq2 = work.tile([128, CW], bf16, tag="sq2", bufs=2)
                    nc.vector.tensor_mul(sq2, raw, raw)
                    ssq = pps.tile([1, CW], f32, tag="ssq")
                    nc.tensor.matmul(ssq, ones_col_bf, sq2, start=True, stop=True)
                    lnm = small.tile([1, CW], f32, tag="lnm", bufs=2)
                    if idx < HL:  # m = ssq/D + eps
                        nc.scalar.activation(lnm, ssq, AF.Ln, bias=eps_one, scale=1.0 / D)
                    else:  # m = ssq + D*eps: folds logit 1/sqrt(D) into k
                        nc.scalar.activation(lnm, ssq, AF.Ln, bias=eps_row)
                    ri = small.tile([1, CW], bf16, tag="ri")
                    nc.scalar.activation(ri, lnm, AF.Exp, scale=-0.5)
                    ris.append(ri)
                # rope (sign folded into sin tables) + normalize
                for idx in range(3):
                    raw, rotr = raws[idx], rotrs[idx]
                    cos_sb, sin_sb = (cq_sb, sq_sb) if idx < HL else (ck_sb, sk_sb)
                    dst = qt_sb[:, idx, csl] if idx < HL else kt_sb[:, csl]
                    rsb = pp.tile([128, CW], f32, tag="bcast", bufs=1, name="rsb")
                    nc.tensor.matmul(rsb, ones_row_bf, ris[idx], start=True, stop=True)
                    t1 = work.tile([128, CW], bf16, tag="t1", bufs=2)
                    nc.vector.tensor_mul(t1, raw, cos_sb[:, csl])
                    t2 = work.tile([128, CW], bf16, tag="t2", bufs=2)
                    nc.vector.tensor_mul(t2, rotr, sin_sb[:, csl])
                    pre = work.tile([128, CW], bf16, tag="pre", bufs=2)
                    nc.vector.tensor_add(pre, t1, t2)
                    nc.vector.tensor_mul(dst, pre, rsb)

            # ---------- phase C: causal attention for one (head, chunk) ----
            def attn(j, ic):
                o_ps = pp.tile([128, CW], f32, tag="ops", name="ops", bufs=1)
                p_acc = work.tile([128, CW], bf16, tag="pacc", bufs=2)
                njt = 4 * ic + 4  # causal: j-tiles 0 .. 4*ic+3
                def qk_exp(jt):
                    t_ = jt - 4 * ic
                    lo = t_ * 128 if t_ > 0 else 0
                    jsl = slice(jt * 128, (jt + 1) * 128)
                    st = pp.tile([128, CW], f32, tag="st", name="st", bufs=2)
                    nc.tensor.matmul(
                        st[:, lo:],
                        kt_sb[:, jsl],
                        qt_sb[:, j, ic * CW + lo : (ic + 1) * CW],
                        start=True, stop=True,
                    )
                    p = work.tile([128, CW], bf16, tag="p", bufs=6, name="p")
                    nc.scalar.activation(p[:, lo:], st[:, lo:], AF.Exp)
                    if t_ >= 0:  # diagonal block: causal mask
                        nc.vector.tensor_mul(p[:, lo:], p[:, lo:], msk_sb[:, t_, lo:])
                    # denominator accumulation right after exp/mask (bf16)
                    if jt == 0:
                        nc.vector.tensor_copy(p_acc, p)
                    else:
                        nc.vector.tensor_add(p_acc[:, lo:], p_acc[:, lo:], p[:, lo:])
                    return (p, lo)

                def av(jt, plo):
                    p, lo = plo
                    mm = dict(start=(jt == 0), stop=(jt == njt - 1))
                    nc.tensor.matmul(o_ps[:, lo:], v_sb[:, jt, :], p[:, lo:], **mm)

                pipe = [qk_exp(0)]
                if njt > 1:
                    pipe.append(qk_exp(1))
                for jt in range(2, njt):
                    cur = qk_exp(jt)
                    av(jt - 2, pipe.pop(0))
                    pipe.append(cur)
                for k_, p_ in enumerate(pipe):
                    av(njt - len(pipe) + k_, p_)
                l_ps = pps.tile([1, CW], f32, tag="ssq", name="lps")
                nc.tensor.matmul(l_ps, ones_col_bf, p_acc, start=True, stop=True)
                linv = small.tile([1, CW], f32, tag="linv", bufs=2)
                nc.vector.reciprocal_approx_fast(linv, l_ps)
                # broadcast 1/l via PE outer product (f32r: 1 cycle/row)
                linv_bf = small.tile([1, CW], bf16, tag="libf", bufs=2)
                nc.vector.tensor_copy(linv_bf, linv)
                lb_ps = pp.tile([128, CW], f32, tag="bcast", bufs=1, name="lbps")
                nc.tensor.matmul(lb_ps, ones_row_bf, linv_bf, start=True, stop=True)
                oc = work.tile([128, CW], bf16, tag="oc", bufs=2, name="oc")
                nc.vector.tensor_copy(oc, o_ps)
                ot = work.tile([128, CW], bf16, tag="ot", bufs=3, name="ot")
                nc.vector.tensor_mul(ot, oc, lb_ps)
                nc.sync.dma_start(
                    a2a_in[j][:, :].rearrange("(r p) s -> p r s", p=128)[
                        :, 2 * ic : 2 * ic + 2, :
                    ],
                    ot.rearrange("p (r s) -> p r s", r=2),
                )

            def a2a(j):
                nc.gpsimd.collective_compute(
                    "AllToAll",
                    mybir.AluOpType.bypass,
                    replica_groups=[list(range(NC))],
                    ins=[a2a_in[j][:, :].opt()],
                    outs=[a2a_out[j][:, :].opt()],
                )

            # ---------- emission: interleave projection + attention --------
            # tiny warmup AllToAll absorbs the first-collective ncfw cost
            # (observed ~11.5us trigger->start delay) while inputs load
            nc.gpsimd.collective_compute(
                "AllToAll",
                mybir.AluOpType.bypass,
                replica_groups=[list(range(NC))],
                ins=[warm_in[:, :].opt()],
                outs=[warm_out[:, :].opt()],
            )
            # second, 64KB warmup: exercises the collective data path at a
            # realistic size so the first real AllToAll runs warm
            nc.gpsimd.collective_compute(
                "AllToAll",
                mybir.AluOpType.bypass,
                replica_groups=[list(range(NC))],
                ins=[warm2_in[:, :].opt()],
                outs=[warm2_out[:, :].opt()],
            )
            phase_a(0)
            phase_a(1)
            attn(0, 0)
            attn(1, 0)
            attn(0, 1)
            attn(1, 1)
            phase_a(2)
            attn(0, 2)
            attn(1, 2)
            phase_a(3)
            attn(0, 3)
            a2a(0)
            attn(1, 3)
            a2a(1)

            # readback per 64KB g-block, on the scalar queue AFTER all exps:
            # a2a_out[j] shard g (g = src core) = global head 2g+j for my
            # 256 tokens, laid out [d(128 rows), 256 tok].
            for j in range(HL):
                for g in range(NC):
                    nc.scalar.dma_start(
                        att_sb[:, j, g, :],
                        a2a_out[j][:, :].rearrange("(g p) s -> p g s", p=128)[
                            :, g, :
                        ],
                    )

            # ---------- phase D: output projection (8 psum banks) ----------
            y_specs = [
                ("big", 3), ("big", 3), ("big", 3), ("st", 2),
                ("st", 2), ("ops", 1), ("bcast", 1),
            ]
            y_ps = [
                pp.tile([128, CW], f32, tag=tg, name=f"yps{i}", bufs=bf)
                for i, (tg, bf) in enumerate(y_specs)
            ]
            y_ps.append(pps.tile([128, CW], f32, tag="ssq", name="yps7"))

            def d_even():  # even heads (ht=2g), all 8 banks
                for g in range(NC):
                    for st in range(2):
                        for oc in range(4):
                            nc.tensor.matmul(
                                y_ps[st * 4 + oc],
                                att_sb[:, 0, g, st * 128 : (st + 1) * 128],
                                wo_sb[:, 2 * g, oc * CW : (oc + 1) * CW],
                                start=(g == 0), stop=False,
                            )

            def d_odd_st(st):  # odd heads, one token-half: frees its banks early
                for g in range(NC):
                    for oc in range(4):
                        nc.tensor.matmul(
                            y_ps[st * 4 + oc],
                            att_sb[:, 1, g, st * 128 : (st + 1) * 128],
                            wo_sb[:, 2 * g + 1, oc * CW : (oc + 1) * CW],
                            start=False, stop=(g == NC - 1),
                        )

            def d_store(st):
                for oc in range(4):
                    yo = work.tile([128, CW], bf16, tag="yo", bufs=3, name="yo")
                    nc.vector.tensor_copy(yo, y_ps[st * 4 + oc])
                    nc.sync.dma_start(
                        out_d[st * 128 : (st + 1) * 128, oc * CW : (oc + 1) * CW],
                        yo,
                    )

            d_even()
            d_odd_st(0)
            d_store(0)
            d_odd_st(1)
            d_store(1)

    nc.compile()
    return nc


def _get_nc(reps: int = 1):
    key = f"nc{reps}"
    if key not in _cache:
        _cache[key] = _build_nc(reps)
    return _cache[key]


def _prep_in_maps(
    hidden_states, cos, sin, Wq, Wk, Wv, Wo, q_norm_scale, k_norm_scale,
    last_norm_scale, attention_mask,
):
    xt = np.asarray(hidden_states, np.float32)[0].T  # [H, S]
    xt_p = np.ascontiguousarray(
        xt.reshape(KT, 128, S).transpose(1, 0, 2)
    ).astype(BF16)  # [128, KT, S]
    wo = np.ascontiguousarray(np.asarray(Wo, np.float32)).astype(BF16)
    cosr = np.asarray(cos, np.float32)[:, 0, :]  # [S, D]
    sinr = np.asarray(sin, np.float32)[:, 0, :]
    # rotate-half sign vector, folded into the sin tables (device does a pure
    # partition rotation by 64)
    sign = np.concatenate([-np.ones(64, np.float32), np.ones(64, np.float32)])

    def rope_tables(scale):
        sc = np.asarray(scale, np.float32)
        c_eff = np.ascontiguousarray(cosr.T * sc[:, None]).astype(BF16)  # [D, S]
        rsc = np.concatenate([sc[64:], sc[:64]])  # scale[(d+64)%128]
        s_eff = sinr.T * (rsc * sign)[:, None]
        return c_eff, np.ascontiguousarray(s_eff).astype(BF16)

    cq, sq = rope_tables(q_norm_scale)
    ck, sk = rope_tables(k_norm_scale)

    msk = np.zeros((D, 4, CW), np.float32)
    jj = np.arange(128)[:, None]
    ii = np.arange(CW)[None, :]
    for t in range(4):
        msk[:, t, :] = (ii >= jj + t * 128).astype(np.float32)
    msk = msk.astype(BF16)
    ident = np.eye(128, dtype=np.float32).astype(BF16)

    def pack_w(w):
        # [H, C] -> [128, KT, C] with w[t*128+p, c] at [p, t, c]
        return np.ascontiguousarray(
            np.asarray(w, np.float32).reshape(KT, 128, -1).transpose(1, 0, 2)
        ).astype(BF16)

    Wq = np.asarray(Wq, np.float32)
    Wk = np.asarray(Wk, np.float32)
    Wv = np.asarray(Wv, np.float32)
    in_maps = []
    for i in range(NC):
        kv = i // 2
        in_maps.append(
            {
                "xt": xt_p,
                "wq": pack_w(Wq[:, i * HL * D : (i + 1) * HL * D]),
                "wk": pack_w(Wk[:, kv * D : (kv + 1) * D]),
                "wv": pack_w(Wv[:, kv * D : (kv + 1) * D]),
                "wo": wo,
                "cq": cq,
                "sq": sq,
                "ck": ck,
                "sk": sk,
                "msk": msk,
                "ident": ident,
            }
        )
    return in_maps


last_results = None


def kernel(**inputs) -> np.ndarray:
    global last_results
    from concourse import bass_utils

    nc = _get_nc()
    in_maps = _prep_in_maps(**inputs)
    res = bass_utils.run_bass_kernel_spmd(nc, in_maps, core_ids=list(range(NC)))
    last_results = res
    parts = [np.asarray(res.results[i]["out"], np.float32) for i in range(NC)]
    y = np.concatenate(parts, axis=0)  # [S, H] pre-norm
    # final RMSNorm is a pure per-token scale: applied host-side (exact),
    # together with the per-channel last_norm_scale
    rms = np.sqrt(np.mean(np.square(y), axis=-1, keepdims=True) + EPS)
    out = (y / rms)[None, :, :]
    return out * np.asarray(inputs["last_norm_scale"], np.float32)[None, None, :]


# revision 26
# speedup vs baseline: 1.0009x; 1.0009x over previous
"""Distributed GQA attention block (RMSNorm-QK + RoPE + causal attention + Wo)
for one TRN2 chip (8 NeuronCores).

Sharding: tensor-parallel over heads. Core i computes q-heads {2i, 2i+1} and
kv-head i//2. Everything on-device is computed transposed ([dim, seq]) so the
hidden/contraction axis lands on SBUF partitions with zero on-device
transposes of X. An AllToAll redistributes the attention output from
head-sharded to sequence-sharded; each core then runs the output projection
for its own 256 tokens. The final RMSNorm is a pure per-token scale, so it is
applied host-side (exactly), together with last_norm_scale.

v8 structure (from v3 via trace-driven iteration; 303us -> ~217us):
- X^T streamed per 512-column chunk through 2 rotating SBUF buffers; Wo is
  resident in its own SBUF space and loaded early on the sync queue, so no
  WAR alias with X and no HBM spike colliding with the AllToAlls.
- Startup split: wq + rope tables pull on the scalar ring while x chunks pull
  on the sync ring; first projection matmul starts ~10us in (8us preamble).
- RMS stats use Ln->Exp (ri = exp(-0.5*ln(m))) on the scalar engine, plus an
  act-table patch steering both onto the one table set that holds ln AND exp
  (natural_log_exp_and_others): activation-table reloads drop 25 -> 2.
- Per-token 1/sqrt broadcasts ([1,CW] -> [128,CW]) ride PE outer products in
  bf16 (1 cycle/row); ri is produced in bf16 directly by the Exp activation.
- Softmax denominator accumulated in bf16 (2x DVE rate); denominator errors
  are per-token scales that the final (host-side) RMSNorm cancels exactly.
- V tiles transposed on the PE (identity matmul) instead of DMA-transpose.
- A tiny warmup AllToAll at kernel start absorbs the first-collective ncfw
  bootstrap (~50us "Invalid" CC op + ~11us trigger->start delay).
- Head-1 attention chunks are interleaved right after their head-0 siblings
  (both ready at the same time), except (1,3): the head-0 AllToAll fires
  ~12us before attention ends and overlaps attn(1,3).
- a2a_out readbacks ride the scalar queue AFTER all exps (structurally
  impossible for phase-D matmuls to be queued ahead of attention tail work),
  per 64KB g-block.
- Phase D uses all 8 PSUM banks: even heads (post a2a#0) accumulate while
  a2a#1 flies; odd-head accumulation continues in the same banks, split by
  token-half so the first half's stores overlap the second half's matmuls.
- fp8 was tried and rejected: with random-sign values the Wo contraction and
  the p-weighted V average inherit elementwise quantization error ~1:1, so
  e4m3's ~4-5% blows the 2e-2 gate (measured 6.5e-2 proj / 3.8e-2 AV).

Numerics: bf16 matmuls with f32 PSUM accumulation; softmax without
max-subtraction (logits are O(1)); causal mask applied multiplicatively
after exp; K normalization folds the 1/sqrt(D) logit scale.
"""

import sys

sys.path.insert(0, "/opt/trn_rl_repo")

import numpy as np
import ml_dtypes

BF16 = ml_dtypes.bfloat16

S = 2048  # sequence length
H = 2048  # hidden
D = 128  # head dim
NH = 16  # query heads
NKV = 4  # kv heads
NC = 8  # cores
HL = NH // NC  # q heads per core = 2
SC = S // NC  # seq per core (output shard) = 256
CH = 4  # seq chunks
CW = 512  # chunk width
KT = H // 128  # contraction tiles = 16
EPS = 1e-6

_cache: dict = {}


def _patch_act_tables():
    """Steer Ln/Exp activations onto the one table set that holds BOTH
    (natural_log_exp_and_others), so the scalar engine never reloads its
    activation table mid-kernel. Set positions (= act_func_set ids) are
    unchanged; only which sets advertise Ln/Exp to the selection pass."""
    from concourse import hw_specs, bacc as _bacc, bass_interp as _bi

    if getattr(hw_specs, "_ant_lnexp_patch", False):
        return
    orig = hw_specs.get_activation_tables

    def patched(arch):
        tabs = orig(arch)
        both = None
        for name, fns in tabs.items():
            names = {f.name for f in fns}
            if "Exp" in names and "Ln" in names:
                both = name
                break
        if both is not None:
            for name in list(tabs):
                if name != both:
                    tabs[name] = {
                        f for f in tabs[name] if f.name not in ("Exp", "Ln")
                    }
        return tabs

    hw_specs.get_activation_tables = patched
    hw_specs._ant_lnexp_patch = True
    for mod in (_bacc, _bi):
        if getattr(mod, "get_activation_tables", None) is not None:
            mod.get_activation_tables = patched


def _build_nc(reps: int = 1):
    import concourse.bass as bass
    import concourse.tile as tile
    from concourse import bacc, mybir

    _patch_act_tables()

    f32 = mybir.dt.float32
    f32r = mybir.dt.float32r
    bf16 = mybir.dt.bfloat16
    AF = mybir.ActivationFunctionType

    nc = bacc.Bacc("TRN2", target_bir_lowering=False, debug=False, num_devices=NC)

    # ---- kernel I/O (per-core shards; replicated where noted) ----
    xt_d = nc.dram_tensor("xt", [128, KT, S], bf16, kind="ExternalInput").ap()
    wq_d = nc.dram_tensor("wq", [128, KT, HL * D], bf16, kind="ExternalInput").ap()
    wk_d = nc.dram_tensor("wk", [128, KT, D], bf16, kind="ExternalInput").ap()
    wv_d = nc.dram_tensor("wv", [128, KT, D], bf16, kind="ExternalInput").ap()
    wo_d = nc.dram_tensor("wo", [H, H], bf16, kind="ExternalInput").ap()
    cq_d = nc.dram_tensor("cq", [D, S], bf16, kind="ExternalInput").ap()
    sq_d = nc.dram_tensor("sq", [D, S], bf16, kind="ExternalInput").ap()
    ck_d = nc.dram_tensor("ck", [D, S], bf16, kind="ExternalInput").ap()
    sk_d = nc.dram_tensor("sk", [D, S], bf16, kind="ExternalInput").ap()
    msk_d = nc.dram_tensor("msk", [D, 4, CW], bf16, kind="ExternalInput").ap()
    id_d = nc.dram_tensor("ident", [128, 128], bf16, kind="ExternalInput").ap()
    out_d = nc.dram_tensor("out", [SC, H], bf16, kind="ExternalOutput").ap()

    with tile.TileContext(nc) as tc:
        with (
            tc.tile_pool(name="singles", bufs=1) as singles,
            tc.tile_pool(name="xs", bufs=2) as xs,  # streamed X chunks
            tc.tile_pool(name="work", bufs=3) as work,
            tc.tile_pool(name="small", bufs=3) as small,
            tc.tile_pool(name="psum", bufs=1, space="PSUM") as pp,
            tc.tile_pool(name="psmall", bufs=1, space="PSUM") as pps,
            tc.tile_pool(name="dram", bufs=1, space="DRAM") as dram,
        ):
            # ---------- resident SBUF tensors ----------
            wq_sb = singles.tile([128, KT, HL * D], bf16)
            wk_sb = singles.tile([128, KT, D], bf16)
            wv_sb = singles.tile([128, KT, D], bf16)
            cq_sb = singles.tile([128, S], bf16)
            sq_sb = singles.tile([128, S], bf16)
            ck_sb = singles.tile([128, S], bf16)
            sk_sb = singles.tile([128, S], bf16)
            msk_sb = singles.tile([128, 4, CW], bf16)
            id_sb = singles.tile([128, 128], bf16)
            wo_sb = singles.tile([128, KT, S], bf16)

            # x chunk stream: 2 rotating buffers
            x_c = [
                xs.tile([128, KT, CW], bf16, tag="xc", name=f"xc{c}")
                for c in range(CH)
            ]
            # scalar queue: wq + rope tables (idle at start); sync queue: x
            # chunks, wk/wv, wo; both rings pull in parallel so the first
            # projection matmul can start ~4us in.
            for gs in [slice(0, 2), slice(2, 4), slice(4, 8), slice(8, 12), slice(12, 16)]:
                nc.scalar.dma_start(wq_sb[:, gs, :], wq_d[:, gs, :])
            for tsl in [slice(0, 2), slice(2, 4), slice(4, 8), slice(8, 12), slice(12, 16)]:
                nc.sync.dma_start(x_c[0][:, tsl, :], xt_d[:, tsl, 0:CW])
            nc.scalar.dma_start(cq_sb, cq_d)
            nc.scalar.dma_start(sq_sb, sq_d)
            nc.scalar.dma_start(ck_sb, ck_d)
            nc.scalar.dma_start(sk_sb, sk_d)
            for g in range(4):
                gs = slice(4 * g, 4 * g + 4)
                nc.sync.dma_start(wk_sb[:, gs, :], wk_d[:, gs, :])
                nc.sync.dma_start(wv_sb[:, gs, :], wv_d[:, gs, :])
            for tp in range(4):
                tsl = slice(4 * tp, 4 * tp + 4)
                nc.sync.dma_start(x_c[1][:, tsl, :], xt_d[:, tsl, CW : 2 * CW])
            nc.sync.dma_start(msk_sb, msk_d)
            nc.sync.dma_start(id_sb, id_d)
            for tp in range(4):
                tsl = slice(4 * tp, 4 * tp + 4)
                nc.sync.dma_start(x_c[2][:, tsl, :], xt_d[:, tsl, 2 * CW : 3 * CW])
            for tp in range(4):
                tsl = slice(4 * tp, 4 * tp + 4)
                nc.sync.dma_start(x_c[3][:, tsl, :], xt_d[:, tsl, 3 * CW : 4 * CW])
            for ht in range(KT):
                nc.sync.dma_start(
                    wo_sb[:, ht, :], wo_d[ht * 128 : (ht + 1) * 128, :]
                )

            ones_col_bf = singles.tile([128, 1], bf16)  # lhsT for partition sums
            nc.vector.memset(ones_col_bf, 1.0)
            ones_row_bf = singles.tile([1, 128], bf16)  # lhsT for row broadcasts
            nc.vector.memset(ones_row_bf, 1.0)
            eps_row = singles.tile([1, 1], f32)  # D*eps for k sumsq
            nc.vector.memset(eps_row, D * EPS)
            eps_one = singles.tile([1, 1], f32)  # eps for q mean-sumsq
            nc.vector.memset(eps_one, EPS)

            qt_sb = singles.tile([128, HL, S], bf16)  # roped Q^T per local head
            kt_sb = singles.tile([128, S], bf16)  # normalized+roped K^T
            v_sb = singles.tile([128, KT, D], bf16)  # V tiles [j-part, jt, d]
            att_sb = singles.tile([128, HL, NC, SC], bf16)  # att^T post-A2A
            a2a_in = [
                dram.tile([S // 2, SC], bf16, name=f"a2ai{j}") for j in range(HL)
            ]
            a2a_out = [
                dram.tile([S // 2, SC], bf16, name=f"a2ao{j}") for j in range(HL)
            ]
            # head-0 redistribution as two contiguous token-half tensors so
            # the first half's collective can land (and un-gate phase D) early
            a2a_in0 = [
                dram.tile([S // 2, SC // 2], bf16, name=f"a2ai0h{h}")
                for h in range(2)
            ]
            a2a_out0 = [
                dram.tile([S // 2, SC // 2], bf16, name=f"a2ao0h{h}")
                for h in range(2)
            ]
            warm_in = dram.tile([NC, 16], bf16, name="warm_in")
            warm_out = dram.tile([NC, 16], bf16, name="warm_out")
            warm2_in = dram.tile([NC * 128, 32], bf16, name="warm2_in")
            warm2_out = dram.tile([NC * 128, 32], bf16, name="warm2_out")

            assert reps == 1

            # ---------- phase A: QKV projection + RMS-stats + RoPE ----------
            def phase_a(c):
                csl = slice(c * CW, (c + 1) * CW)
                xc = x_c[c]
                q_ps = [
                    pp.tile([128, CW], f32, tag="big", name=f"qps{j}", bufs=3)
                    for j in range(HL)
                ]
                for ht in range(KT):
                    mm = dict(start=(ht == 0), stop=(ht == KT - 1))
                    for j in range(HL):
                        nc.tensor.matmul(
                            q_ps[j], wq_sb[:, ht, j * D : (j + 1) * D],
                            xc[:, ht, :], **mm
                        )
                raws, rotrs = [], []

                def raw_rot(ps, name):
                    raw = work.tile([128, CW], bf16, tag="raw", bufs=4, name=name)
                    nc.vector.tensor_copy(raw, ps)
                    rotr = work.tile([128, CW], bf16, tag="rotr", bufs=3)
                    nc.gpsimd.dma_start(rotr[0:64, :], raw[64:128, :])
                    nc.gpsimd.dma_start(rotr[64:128, :], raw[0:64, :])
                    raws.append(raw)
                    rotrs.append(rotr)

                raw_rot(q_ps[0], "rawq0")
                raw_rot(q_ps[1], "rawq1")
                k_ps = pp.tile([128, CW], f32, tag="big", bufs=3)
                v_ps = pp.tile([128, CW], f32, tag="big", bufs=3)
                for ht in range(KT):
                    mm = dict(start=(ht == 0), stop=(ht == KT - 1))
                    nc.tensor.matmul(k_ps, wk_sb[:, ht, :], xc[:, ht, :], **mm)
                    nc.tensor.matmul(v_ps, wv_sb[:, ht, :], xc[:, ht, :], **mm)
                raw_rot(k_ps, "rawk")
                # V: copy psum -> SBUF (transposed layout), then PE-transpose
                # each 128x128 tile into v_sb
                vt = work.tile([128, CW], bf16, tag="vt", bufs=2, name="vt")
                nc.vector.tensor_copy(vt, v_ps)
                for t_ in range(4):
                    jt = 4 * c + t_
                    vT_ps = pp.tile([128, 128], bf16, tag="st", bufs=2, name="vTps")
                    nc.tensor.transpose(
                        vT_ps, vt[:, t_ * 128 : (t_ + 1) * 128], id_sb
                    )
                    nc.vector.tensor_copy(v_sb[:, jt, :], vT_ps)
                # RMS stats: ri = exp(-0.5 * ln(m)); ln+exp share one ACT table
                ris = []
                for idx in range(3):
                    raw = raws[idx]
                    sq2 = work.tile([128, CW], bf16, tag="sq2", bufs=2)
                    nc.vector.tensor_mul(sq2, raw, raw)
                    ssq = pps.tile([1, CW], f32, tag="ssq")
                    nc.tensor.matmul(ssq, ones_col_bf, sq2, start=True, stop=True)
                    lnm = small.tile([1, CW], f32, tag="lnm", bufs=2)
                    if idx < HL:  # m = ssq/D + eps
                        nc.scalar.activation(lnm, ssq, AF.Ln, bias=eps_one, scale=1.0 / D)
                    else:  # m = ssq + D*eps: folds logit 1/sqrt(D) into k
                        nc.scalar.activation(lnm, ssq, AF.Ln, bias=eps_row)
                    ri = small.tile([1, CW], bf16, tag="ri")
                    nc.scalar.activation(ri, lnm, AF.Exp, scale=-0.5)
                    ris.append(ri)
                # rope (sign folded into sin tables) + normalize
                for idx in range(3):
                    raw, rotr = raws[idx], rotrs[idx]
                    cos_sb, sin_sb = (cq_sb, sq_sb) if idx < HL else (ck_sb, sk_sb)
                    dst = qt_sb[:, idx, csl] if idx < HL else kt_sb[:, csl]
                    rsb = pp.tile([128, CW], f32, tag="bcast", bufs=1, name="rsb")
                    nc.tensor.matmul(rsb, ones_row_bf, ris[idx], start=True, stop=True)
                    t1 = work.tile([128, CW], bf16, tag="t1", bufs=2)
                    nc.vector.tensor_mul(t1, raw, cos_sb[:, csl])
                    t2 = work.tile([128, CW], bf16, tag="t2", bufs=2)
                    nc.vector.tensor_mul(t2, rotr, sin_sb[:, csl])
                    pre = work.tile([128, CW], bf16, tag="pre", bufs=2)
                    nc.vector.tensor_add(pre, t1, t2)
                    nc.vector.tensor_mul(dst, pre, rsb)

            # ---------- phase C: causal attention for one (head, chunk) ----
            def attn(j, ic):
                o_ps = pp.tile([128, CW], f32, tag="ops", name="ops", bufs=1)
                p_acc = work.tile([128, CW], bf16, tag="pacc", bufs=2)
                njt = 4 * ic + 4  # causal: j-tiles 0 .. 4*ic+3
                def qk_exp(jt):
                    t_ = jt - 4 * ic
                    lo = t_ * 128 if t_ > 0 else 0
                    jsl = slice(jt * 128, (jt + 1) * 128)
                    st = pp.tile([128, CW], f32, tag="st", name="st", bufs=2)
                    nc.tensor.matmul(
                        st[:, lo:],
                        kt_sb[:, jsl],
                        qt_sb[:, j, ic * CW + lo : (ic + 1) * CW],
                        start=True, stop=True,
                    )
                    p = work.tile([128, CW], bf16, tag="p", bufs=6, name="p")
                    nc.scalar.activation(p[:, lo:], st[:, lo:], AF.Exp)
                    if t_ >= 0:  # diagonal block: causal mask
                        nc.vector.tensor_mul(p[:, lo:], p[:, lo:], msk_sb[:, t_, lo:])
                    # denominator accumulation right after exp/mask (bf16)
                    if jt == 0:
                        nc.vector.tensor_copy(p_acc, p)
                    else:
                        nc.vector.tensor_add(p_acc[:, lo:], p_acc[:, lo:], p[:, lo:])
                    return (p, lo)

                def av(jt, plo):
                    p, lo = plo
                    mm = dict(start=(jt == 0), stop=(jt == njt - 1))
                    nc.tensor.matmul(o_ps[:, lo:], v_sb[:, jt, :], p[:, lo:], **mm)

                pipe = [qk_exp(0)]
                if njt > 1:
                    pipe.append(qk_exp(1))
                for jt in range(2, njt):
                    cur = qk_exp(jt)
                    av(jt - 2, pipe.pop(0))
                    pipe.append(cur)
                for k_, p_ in enumerate(pipe):
                    av(njt - len(pipe) + k_, p_)
                l_ps = pps.tile([1, CW], f32, tag="ssq", name="lps")
                nc.tensor.matmul(l_ps, ones_col_bf, p_acc, start=True, stop=True)
                linv = small.tile([1, CW], f32, tag="linv", bufs=2)
                nc.vector.reciprocal_approx_fast(linv, l_ps)
                # broadcast 1/l via PE outer product (f32r: 1 cycle/row)
                linv_bf = small.tile([1, CW], bf16, tag="libf", bufs=2)
                nc.vector.tensor_copy(linv_bf, linv)
                lb_ps = pp.tile([128, CW], f32, tag="bcast", bufs=1, name="lbps")
                nc.tensor.matmul(lb_ps, ones_row_bf, linv_bf, start=True, stop=True)
                oc = work.tile([128, CW], bf16, tag="oc", bufs=2, name="oc")
                nc.vector.tensor_copy(oc, o_ps)
                ot = work.tile([128, CW], bf16, tag="ot", bufs=3, name="ot")
                nc.vector.tensor_mul(ot, oc, lb_ps)
                if j == 0:
                    for h in range(2):
                        nc.sync.dma_start(
                            a2a_in0[h][:, :].rearrange("(r p) s -> p r s", p=128)[
                                :, 2 * ic : 2 * ic + 2, :
                            ],
                            ot.rearrange("p (r s) -> p r s", r=2)[
                                :, :, h * 128 : (h + 1) * 128
                            ],
                        )
                else:
                    nc.sync.dma_start(
                        a2a_in[j][:, :].rearrange("(r p) s -> p r s", p=128)[
                            :, 2 * ic : 2 * ic + 2, :
                        ],
                        ot.rearrange("p (r s) -> p r s", r=2),
                    )

            def a2a(j):
                nc.gpsimd.collective_compute(
                    "AllToAll",
                    mybir.AluOpType.bypass,
                    replica_groups=[list(range(NC))],
                    ins=[a2a_in[j][:, :].opt()],
                    outs=[a2a_out[j][:, :].opt()],
                )

            def a2a0h(h):
                nc.gpsimd.collective_compute(
                    "AllToAll",
                    mybir.AluOpType.bypass,
                    replica_groups=[list(range(NC))],
                    ins=[a2a_in0[h][:, :].opt()],
                    outs=[a2a_out0[h][:, :].opt()],
                )

            # ---------- emission: interleave projection + attention --------
            # tiny warmup AllToAll absorbs the first-collective ncfw cost
            # (observed ~11.5us trigger->start delay) while inputs load
            nc.gpsimd.collective_compute(
                "AllToAll",
                mybir.AluOpType.bypass,
                replica_groups=[list(range(NC))],
                ins=[warm_in[:, :].opt()],
                outs=[warm_out[:, :].opt()],
            )
            # second, 64KB warmup: exercises the collective data path at a
            # realistic size so the first real AllToAll runs warm
            nc.gpsimd.collective_compute(
                "AllToAll",
                mybir.AluOpType.bypass,
                replica_groups=[list(range(NC))],
                ins=[warm2_in[:, :].opt()],
                outs=[warm2_out[:, :].opt()],
            )
            phase_a(0)
            phase_a(1)
            attn(0, 0)
            attn(1, 0)
            attn(0, 1)
            attn(1, 1)
            phase_a(2)
            attn(0, 2)
            attn(1, 2)
            phase_a(3)
            attn(0, 3)
            # head-0 a2a split by token-half: the st=0 half lands first and
            # un-gates the even/st=0 quarter of phase D while st=1 flies
            a2a0h(0)
            a2a0h(1)
            attn(1, 3)
            a2a(1)

            # readback per 64KB g-block, on the scalar queue AFTER all exps:
            # a2a_out[j] shard g (g = src core) = global head 2g+j for my
            # 256 tokens, laid out [d(128 rows), 256 tok].
            for st in range(2):
                for g in range(NC):
                    nc.scalar.dma_start(
                        att_sb[:, 0, g, st * 128 : (st + 1) * 128],
                        a2a_out0[st][:, :].rearrange("(g p) s -> p g s", p=128)[
                            :, g, :
                        ],
                    )
            for g in range(NC):
                nc.scalar.dma_start(
                    att_sb[:, 1, g, :],
                    a2a_out[1][:, :].rearrange("(g p) s -> p g s", p=128)[
                        :, g, :
                    ],
                )

            # ---------- phase D: output projection (8 psum banks) ----------
            y_specs = [
                ("big", 3), ("big", 3), ("big", 3), ("st", 2),
                ("st", 2), ("ops", 1), ("bcast", 1),
            ]
            y_ps = [
                pp.tile([128, CW], f32, tag=tg, name=f"yps{i}", bufs=bf)
                for i, (tg, bf) in enumerate(y_specs)
            ]
            y_ps.append(pps.tile([128, CW], f32, tag="ssq", name="yps7"))

            def d_even_st(st):  # even heads (ht=2g), one token-half
                for g in range(NC):
                    for oc in range(4):
                        nc.tensor.matmul(
                            y_ps[st * 4 + oc],
                            att_sb[:, 0, g, st * 128 : (st + 1) * 128],
                            wo_sb[:, 2 * g, oc * CW : (oc + 1) * CW],
                            start=(g == 0), stop=False,
                        )

            def d_odd_st(st):  # odd heads, one token-half: frees its banks early
                for g in range(NC):
                    for oc in range(4):
                        nc.tensor.matmul(
                            y_ps[st * 4 + oc],
                            att_sb[:, 1, g, st * 128 : (st + 1) * 128],
                            wo_sb[:, 2 * g + 1, oc * CW : (oc + 1) * CW],
                            start=False, stop=(g == NC - 1),
                        )

            def d_store(st):
                for oc in range(4):
                    yo = work.tile([128, CW], bf16, tag="yo", bufs=3, name="yo")
                    nc.vector.tensor_copy(yo, y_ps[st * 4 + oc])
                    nc.sync.dma_start(
                        out_d[st * 128 : (st + 1) * 128, oc * CW : (oc + 1) * CW],
                        yo,
                    )

            d_even_st(0)
            d_even_st(1)
            d_odd_st(0)
            d_store(0)
            d_odd_st(1)
            d_store(1)

    nc.compile()
    return nc


def _get_nc(reps: int = 1):
    key = f"nc{reps}"
    if key not in _cache:
        _cache[key] = _build_nc(reps)
    return _cache[key]


def _prep_in_maps(
    hidden_states, cos, sin, Wq, Wk, Wv, Wo, q_norm_scale, k_norm_scale,
    last_norm_scale, attention_mask,
):
    xt = np.asarray(hidden_states, np.float32)[0].T  # [H, S]
    xt_p = np.ascontiguousarray(
        xt.reshape(KT, 128, S).transpose(1, 0, 2)
    ).astype(BF16)  # [128, KT, S]
    wo = np.ascontiguousarray(np.asarray(Wo, np.float32)).astype(BF16)
    cosr = np.asarray(cos, np.float32)[:, 0, :]  # [S, D]
    sinr = np.asarray(sin, np.float32)[:, 0, :]
    # rotate-half sign vector, folded into the sin tables (device does a pure
    # partition rotation by 64)
    sign = np.concatenate([-np.ones(64, np.float32), np.ones(64, np.float32)])

    def rope_tables(scale):
        sc = np.asarray(scale, np.float32)
        c_eff = np.ascontiguousarray(cosr.T * sc[:, None]).astype(BF16)  # [D, S]
        rsc = np.concatenate([sc[64:], sc[:64]])  # scale[(d+64)%128]
        s_eff = sinr.T * (rsc * sign)[:, None]
        return c_eff, np.ascontiguousarray(s_eff).astype(BF16)

    cq, sq = rope_tables(q_norm_scale)
    ck, sk = rope_tables(k_norm_scale)

    msk = np.zeros((D, 4, CW), np.float32)
    jj = np.arange(128)[:, None]
    ii = np.arange(CW)[None, :]
    for t in range(4):
        msk[:, t, :] = (ii >= jj + t * 128).astype(np.float32)
    msk = msk.astype(BF16)
    ident = np.eye(128, dtype=np.float32).astype(BF16)

    def pack_w(w):
        # [H, C] -> [128, KT, C] with w[t*128+p, c] at [p, t, c]
        return np.ascontiguousarray(
            np.asarray(w, np.float32).reshape(KT, 128, -1).transpose(1, 0, 2)
        ).astype(BF16)

    Wq = np.asarray(Wq, np.float32)
    Wk = np.asarray(Wk, np.float32)
    Wv = np.asarray(Wv, np.float32)
    in_maps = []
    for i in range(NC):
        kv = i // 2
        in_maps.append(
            {
                "xt": xt_p,
                "wq": pack_w(Wq[:, i * HL * D : (i + 1) * HL * D]),
                "wk": pack_w(Wk[:, kv * D : (kv + 1) * D]),
                "wv": pack_w(Wv[:, kv * D : (kv + 1) * D]),
                "wo": wo,
                "cq": cq,
                "sq": sq,
                "ck": ck,
                "sk": sk,
                "msk": msk,
                "ident": ident,
            }
        )
    return in_maps


last_results = None


def kernel(**inputs) -> np.ndarray:
    global last_results
    from concourse import bass_utils

    nc = _get_nc()
    in_maps = _prep_in_maps(**inputs)
    res = bass_utils.run_bass_kernel_spmd(nc, in_maps, core_ids=list(range(NC)))
    last_results = res
    parts = [np.asarray(res.results[i]["out"], np.float32) for i in range(NC)]
    y = np.concatenate(parts, axis=0)  # [S, H] pre-norm
    # final RMSNorm is a pure per-token scale: applied host-side (exact),
    # together with the per-channel last_norm_scale
    rms = np.sqrt(np.mean(np.square(y), axis=-1, keepdims=True) + EPS)
    out = (y / rms)[None, :, :]
    return out * np.asarray(inputs["last_norm_scale"], np.float32)[None, None, :]
